# revision 7
# baseline (speedup 1.0000x reference)
"""Trainium2 Bass kernel for nn_CharRNN: 4-layer residual char-LSTM.

Strategy (per core; data-parallel over batch, 8 examples/core):
  - All non-recurrent math is hoisted into bulk matmul phases that produce a
    per-step PRE stream in DRAM:
        PRE_l[t] = ex_{l-1}[t] @ U_l + ex_l[t-1] @ W_l + b_l + [t==0] h0_l @ W_l
    where ex_l = tanh(embT @ V_l + vb_l) is the residual stream (ex_0 = 0) and
    emb comes from a one-hot matmul (host builds the one-hot encoding of xb).
  - The recurrent scan runs as a 4-layer wavefront: at tick tau layer l
    processes t = tau - l.  Per tick, each layer does (in its own 32-column
    group of the PE array, so the four layers' matmuls run concurrently):
        g = PRE(identity-mm) + hrec_{l-1}[t] @ U_l + hrec_l[t-1] @ W_l
    gates are activated/combined with single instructions covering all four
    layers at once (each layer's gates live on partition group 32l), and
    hrec = sig(o)*tanh(c) is PE-transposed back into lhsT layout (4 slots,
    cycling mod 4).
  - logits[t] = sum_l hrec_l[t] @ Why_l is accumulated in-loop (lagged by 3
    ticks so all layers are ready); the ex_l @ Why_l part is bulk-precomputed
    and merged in a tiny final phase.
  - Gate columns are permuted to (i, f, o, g) and padded into a 1024-wide
    layout: i,f at [0:400], o,g at [512:912] (PSUM bank-aligned).
  - The wavefront runs over a padded rectangle tau in [0, T+3); PRE pad rows
    use f=+40, i=-40, o=-40 so out-of-range ticks preserve c and keep hrec=0.
"""
import os
import sys
import contextlib

import numpy as np

sys.path.insert(0, os.path.dirname(os.path.abspath(__file__)))

import concourse.bass as bass
import concourse.tile as tile
from concourse import mybir
from concourse.bass_utils import run_bass_kernel_spmd
from tile_fix import split_excess_waits

F32 = mybir.dt.float32
AF = mybir.ActivationFunctionType
ALU = mybir.AluOpType

V, D, H, L, B, T = 96, 24, 200, 4, 64, 512
NCORES = 8
BS = B // NCORES            # 8 examples per core
G = 4 * H                   # 800
GP = 1024                   # padded gate row (psum bank aligned)
TB = T * BS                 # 4096 (t-major: row j = t*BS + b)
KA = 128
KB = H - KA                 # H split for lhsT partition chunks
PADF = 3                    # wavefront lead-in pad rows in PRE
UNROLL = 8
PRE_ROWS = T + 2 * PADF + 2 * UNROLL  # covers loop-tail prefetch reads
NT = T + L - 1              # 515 ticks
N_ITER = (NT + UNROLL - 1) // UNROLL
NTICKS = N_ITER * UNROLL    # 520

# gate permutation: original order (i,f,g,o) -> (i,f,o,g)
PERM = np.concatenate([np.arange(0, 2 * H), np.arange(3 * H, 4 * H),
                       np.arange(2 * H, 3 * H)])


def _pad_gate_row(x):
    """[..., 800] (i,f,o,g order) -> [..., 1024] padded: [0:400]=i,f,
    [512:912]=o,g."""
    out = np.zeros(x.shape[:-1] + (GP,), np.float32)
    out[..., 0:400] = x[..., 0:400]
    out[..., 512:912] = x[..., 400:800]
    return out


def build_nc(nT=NT, static_scan=False):
    """Build the SPMD Bass program (same program on all 8 cores)."""
    nc = bass.Bass()
    dp = lambda n, s: nc.declare_dram_parameter(n, s, F32, isOutput=False)

    oh_T = dp("oh_T", [V, TB])            # one-hot(xb) transposed
    c_mat = dp("c_mat", [V, D])           # embedding table C (lhsT)
    emb_aug = dp("emb_aug", [2, TB])      # [ones; sel0] rows for embT
    ex_aug = dp("ex_aug", [2, TB + BS])   # [ones; sel0] rows for ex tiles
    u0_aug = dp("u0_aug", [D + 2, GP])    # U_in | bias | h0@W rows
    u_a = dp("u_a", [3, KA, GP])          # Uh rows 0:128 per hidden layer
    u_b = dp("u_b", [3, KB, GP])          # Uh rows 128:200
    w_a = dp("w_a", [L, KA, GP])          # W rows 0:128 per layer
    w_b = dp("w_b", [L, KB + 2, GP])      # W rows 128:200 | bias | h0@W
    vh_w = dp("vh_w", [3, D, H])          # Vh (lhsT for ex)
    vh_b = dp("vh_b", [3, H])             # Vh bias (per-partition ACT bias)
    why_a = dp("why_a", [L, KA, V])
    why_b = dp("why_b", [L, KB + 1, V])
    c0m = dp("c0m", [128, H])             # merged c0 (rows 32l..32l+BS)
    i8r = dp("i8r", [128, BS])            # identity replicated at 32l
    pad_blk = dp("pad_blk", [2 * PADF + 2 * UNROLL, BS, GP])  # gate pad rows

    logits_out = nc.declare_dram_parameter("logits", [T, BS, V], F32,
                                           isOutput=True)

    # internal DRAM
    pre_dram = [nc.dram_tensor(f"pre{l}", [PRE_ROWS, BS, GP], F32)
                for l in range(L)]
    lrec_dram = nc.dram_tensor("lrec", [NTICKS + UNROLL, BS, V], F32)
    lex_dram = nc.dram_tensor("lex", [TB, V], F32)

    NCH = TB // 128  # 32 column chunks of 128 (=16 timesteps each)

    with tile.TileContext(nc) as tc:
        ctx = contextlib.ExitStack()
        with ctx:
            persist = ctx.enter_context(tc.tile_pool(name="persist", bufs=1))

            # ---- persistent tiles (used across phases) ----
            i8_s = persist.tile([128, BS], F32)
            nc.sync.dma_start(out=i8_s, in_=i8r[:])
            w_a_s = [persist.tile([KA, GP], F32, name=f"wa{l}") for l in range(L)]
            w_b_s = [persist.tile([KB + 2, GP], F32, name=f"wb{l}") for l in range(L)]
            for l in range(L):
                nc.sync.dma_start(out=w_a_s[l], in_=w_a[l])
                nc.sync.dma_start(out=w_b_s[l], in_=w_b[l])
            u_a_s = [persist.tile([KA, GP], F32, name=f"ua{l}") for l in range(3)]
            u_b_s = [persist.tile([KB, GP], F32, name=f"ub{l}") for l in range(3)]
            for l in range(3):
                nc.sync.dma_start(out=u_a_s[l], in_=u_a[l])
                nc.sync.dma_start(out=u_b_s[l], in_=u_b[l])
            why_a_s = [persist.tile([KA, V], F32, name=f"ya{l}") for l in range(L)]
            why_b_s = [persist.tile([KB + 1, V], F32, name=f"yb{l}") for l in range(L)]
            for l in range(L):
                nc.sync.dma_start(out=why_a_s[l], in_=why_a[l])
                nc.sync.dma_start(out=why_b_s[l], in_=why_b[l])

            # embT (+aug rows): rows 0:24 = C^T @ onehotT, rows 24:26 = aug
            embt_s = persist.tile([D + 2, TB], F32, name="embt")
            nc.sync.dma_start(out=embt_s[D : D + 2, :], in_=emb_aug[:])

            # ================= P1: embeddings =================
            with tc.tile_pool(name="p1", bufs=2) as p1pool, \
                 tc.tile_pool(name="p1ps", bufs=2, space="PSUM") as p1ps:
                c_s = p1pool.tile([V, D], F32)
                nc.sync.dma_start(out=c_s, in_=c_mat[:])
                oh_s = p1pool.tile([V, TB], F32)
                nc.sync.dma_start(out=oh_s, in_=oh_T[:])
                for m in range(TB // 512):
                    ps = p1ps.tile([D, 512], F32, name="embps")
                    nc.tensor.matmul(ps, c_s, oh_s[:, m * 512:(m + 1) * 512],
                                     start=True, stop=True)
                    nc.vector.tensor_copy(embt_s[0:D, m * 512:(m + 1) * 512], ps)

            # ================= P2+P3: ex streams, PRE, logits_ex ==========
            # layer 0 PRE: embT_aug @ u0_aug
            u0_s = persist.tile([D + 2, GP], F32, name="u0")
            nc.sync.dma_start(out=u0_s, in_=u0_aug[:])

            with tc.tile_pool(name="p3", bufs=3) as p3pool, \
                 tc.tile_pool(name="p3ex", bufs=1) as p3ex, \
                 tc.tile_pool(name="p3ps", bufs=2, space="PSUM") as p3ps, \
                 tc.tile_pool(name="p3ps2", bufs=1, space="PSUM") as p3ps2:
                # pad rows for every layer's PRE (lead-in + tail)
                ntail = PRE_ROWS - (T + PADF)
                for l in range(L):
                    nc.sync.dma_start(out=pre_dram[l][0:PADF],
                                      in_=pad_blk[0:PADF])
                    nc.sync.dma_start(out=pre_dram[l][T + PADF:PRE_ROWS],
                                      in_=pad_blk[0:ntail])

                for m in range(NCH):
                    sl = slice(m * 128, (m + 1) * 128)
                    ps0 = p3ps.tile([128, 512], F32, name="pre_ps0")
                    ps1 = p3ps.tile([128, 512], F32, name="pre_ps1")
                    nc.tensor.matmul(ps0[:, 0:400], embt_s[:, sl], u0_s[:, 0:400],
                                     start=True, stop=True)
                    nc.tensor.matmul(ps1[:, 0:400], embt_s[:, sl], u0_s[:, 512:912],
                                     start=True, stop=True)
                    st = p3pool.tile([128, GP], F32, name="pre_stage")
                    nc.vector.tensor_copy(st[:, 0:400], ps0[:, 0:400])
                    nc.scalar.copy(st[:, 512:912], ps1[:, 0:400])
                    nc.sync.dma_start(
                        out=pre_dram[0][:].rearrange("t b g -> (t b) g")[
                            PADF * BS + m * 128 : PADF * BS + (m + 1) * 128],
                        in_=st)

                # ex buffers for two adjacent layers (rolling)
                ex_a_s = [p3ex.tile([KA, TB + BS], F32, name=f"exa{j}",
                                    tag=f"exa{j}") for j in range(2)]
                ex_b_s = [p3ex.tile([KB + 2, TB + BS], F32, name=f"exb{j}",
                                    tag=f"exb{j}") for j in range(2)]
                vba_s = p3pool.tile([KA, 1], F32, name="vba")
                vbb_s = p3pool.tile([KB, 1], F32, name="vbb")

                # logits_ex accumulator in DRAM via sbuf staging
                lex_acc = persist.tile([128, NCH, V], F32, name="lexacc")

                for li in range(3):          # hidden layer li+1's ex
                    cur, prv = li % 2, (li + 1) % 2
                    exa, exb = ex_a_s[cur], ex_b_s[cur]
                    # aug rows + zero prefix
                    nc.sync.dma_start(out=exb[KB : KB + 2, :], in_=ex_aug[:])
                    nc.vector.memset(exa[:, 0:BS], 0.0)
                    nc.vector.memset(exb[0:KB, 0:BS], 0.0)
                    vb_col = vh_b[li].rearrange("(h o) -> h o", o=1)
                    nc.sync.dma_start(out=vba_s, in_=vb_col[0:KA])
                    nc.sync.dma_start(out=vbb_s, in_=vb_col[KA:H])
                    vh_s = p3pool.tile([D, H], F32, name="vh")
                    nc.sync.dma_start(out=vh_s, in_=vh_w[li])
                    # ex = tanh(Vh^T @ embT + vb)
                    for m in range(TB // 512):
                        msl = slice(m * 512, (m + 1) * 512)
                        psa = p3ps2.tile([KA, 512], F32, name="exps0")
                        psb = p3ps2.tile([KB, 512], F32, name="exps1")
                        nc.tensor.matmul(psa, vh_s[:, 0:KA], embt_s[0:D, msl],
                                         start=True, stop=True)
                        nc.tensor.matmul(psb, vh_s[:, KA:H], embt_s[0:D, msl],
                                         start=True, stop=True)
                        osl = slice(BS + m * 512, BS + (m + 1) * 512)
                        nc.scalar.activation(exa[:, osl], psa, AF.Tanh,
                                             bias=vba_s)
                        nc.scalar.activation(exb[0:KB, osl], psb, AF.Tanh,
                                             bias=vbb_s)

                    # PRE_{li+1} = ex_li @ U + ex_{li+1}[t-1] @ W + aug
                    # (ex_li is zero for li==0 -> U-term = 0, skip)
                    for m in range(NCH):
                        csl = slice(m * 128, (m + 1) * 128)          # shifted
                        usl = slice(BS + m * 128, BS + (m + 1) * 128)  # unshift
                        ps0 = p3ps.tile([128, 512], F32, name="pre_ps0")
                        ps1 = p3ps.tile([128, 512], F32, name="pre_ps1")
                        # W-term (shifted view, includes bias/h0W aug rows)
                        nc.tensor.matmul(ps0[:, 0:400], exa[:, csl],
                                         w_a_s[li + 1][:, 0:400],
                                         start=True, stop=False)
                        nc.tensor.matmul(ps1[:, 0:400], exa[:, csl],
                                         w_a_s[li + 1][:, 512:912],
                                         start=True, stop=False)
                        lastw = li == 0
                        nc.tensor.matmul(ps0[:, 0:400], exb[:, csl],
                                         w_b_s[li + 1][:, 0:400],
                                         start=False, stop=lastw)
                        nc.tensor.matmul(ps1[:, 0:400], exb[:, csl],
                                         w_b_s[li + 1][:, 512:912],
                                         start=False, stop=lastw)
                        if li > 0:
                            pexa, pexb = ex_a_s[prv], ex_b_s[prv]
                            nc.tensor.matmul(ps0[:, 0:400], pexa[:, usl],
                                             u_a_s[li][:, 0:400],
                                             start=False, stop=False)
                            nc.tensor.matmul(ps1[:, 0:400], pexa[:, usl],
                                             u_a_s[li][:, 512:912],
                                             start=False, stop=False)
                            nc.tensor.matmul(ps0[:, 0:400], pexb[0:KB, usl],
                                             u_b_s[li][:, 0:400],
                                             start=False, stop=True)
                            nc.tensor.matmul(ps1[:, 0:400], pexb[0:KB, usl],
                                             u_b_s[li][:, 512:912],
                                             start=False, stop=True)
                        st = p3pool.tile([128, GP], F32, name="pre_stage")
                        nc.vector.tensor_copy(st[:, 0:400], ps0[:, 0:400])
                        nc.scalar.copy(st[:, 512:912], ps1[:, 0:400])
                        nc.sync.dma_start(
                            out=pre_dram[li + 1][:].rearrange(
                                "t b g -> (t b) g")[
                                PADF * BS + m * 128 : PADF * BS + (m + 1) * 128],
                            in_=st)

                    # logits_ex += ex_{li+1} @ Why_{li+1}  (+ b_y once)
                    for m in range(NCH):
                        usl = slice(BS + m * 128, BS + (m + 1) * 128)
                        psl = p3ps2.tile([128, V], F32, name="lex_ps")
                        nc.tensor.matmul(psl, exa[:, usl], why_a_s[li + 1],
                                         start=True, stop=False)
                        # ones aug row (at KB) x b_y row folds in the bias
                        # (why_b row KB is b_y for layer 1, zero otherwise)
                        nc.tensor.matmul(psl, exb[0:KB + 1, usl],
                                         why_b_s[li + 1][0:KB + 1],
                                         start=False, stop=True)
                        if li == 0:
                            nc.vector.tensor_copy(lex_acc[:, m, :], psl)
                        else:
                            nc.vector.tensor_add(lex_acc[:, m, :],
                                                 lex_acc[:, m, :], psl)

                for m in range(NCH):
                    nc.sync.dma_start(
                        out=lex_dram[m * 128:(m + 1) * 128], in_=lex_acc[:, m, :])

            # ================= P4: wavefront scan =================
            scan = ctx.enter_context(tc.tile_pool(name="scan", bufs=1))
            c_s = scan.tile([128, H], F32, name="c_state")
            nc.sync.dma_start(out=c_s, in_=c0m[:])
            # hrecT slots: 4 per layer (cycling mod 4), zero-initialized
            slots_a = [[scan.tile([KA, BS], F32, name=f"sa{l}_{s}")
                        for s in range(4)] for l in range(L)]
            slots_b = [[scan.tile([KB, BS], F32, name=f"sb{l}_{s}")
                        for s in range(4)] for l in range(L)]
            for l in range(L):
                for s in range(4):
                    nc.vector.memset(slots_a[l][s], 0.0)
                    nc.vector.memset(slots_b[l][s], 0.0)

            # PRE staging: one tile per u (layer on partition group 32l),
            # A set = ticks 8i+u (u<4), B set = ticks 8i+u (u>=4)
            pre_t = [scan.tile([128, GP], F32, name=f"pret{u}")
                     for u in range(UNROLL)]
            out_stage = [scan.tile([BS, V], F32, name=f"ostg{u}")
                         for u in range(UNROLL)]

            work = ctx.enter_context(tc.tile_pool(name="work", bufs=2))
            gps_pool = ctx.enter_context(
                tc.tile_pool(name="gps", bufs=2, space="PSUM"))
            tps_pool = ctx.enter_context(
                tc.tile_pool(name="tps", bufs=1, space="PSUM"))
            lps_pool = ctx.enter_context(
                tc.tile_pool(name="lps", bufs=2, space="PSUM"))

            pre_flat = [pre_dram[l][:].rearrange("t b g -> (t b) g")
                        for l in range(L)]

            def stage_pre(iv, u):
                """DMA PRE rows for tick tau=iv+u into pre_t[u] (all layers)."""
                for l in range(L):
                    # row = (tau - l + PADF) * BS
                    row = (iv + (u - l + PADF)) * BS
                    nc.sync.dma_start(
                        out=pre_t[u][32 * l : 32 * l + BS, :],
                        in_=pre_flat[l][bass.ds(row, BS)])

            def emit_tick(iv, u):
                """One wavefront tick tau = iv + u (iv multiple of UNROLL)."""
                sw = [(u - l) % 4 for l in range(L)]      # slot written (t%4)
                sr = [(u - l - 1) % 4 for l in range(L)]  # slot read (t-1)%4
                g_ps = gps_pool.tile([128, GP], F32, name="g_ps")
                for l in range(L):
                    r = slice(32 * l, 32 * l + BS)
                    tp = (0, 32 * l)
                    tpi = (32 * l, 32 * l)  # identity mm: K rows also at 32l
                    # PRE preload (identity mm)
                    nc.tensor.matmul(g_ps[r, 0:400], i8_s[r], pre_t[u][r, 0:400],
                                     start=True, stop=False, tile_position=tpi)
                    nc.tensor.matmul(g_ps[r, 512:912], i8_s[r],
                                     pre_t[u][r, 512:912],
                                     start=True, stop=False, tile_position=tpi)
                    # hrec_{l-1}[t] @ U_l
                    if l > 0:
                        ua, ub = u_a_s[l - 1], u_b_s[l - 1]
                        pa, pb = slots_a[l - 1][sw[l]], slots_b[l - 1][sw[l]]
                        nc.tensor.matmul(g_ps[r, 0:400], pa, ua[:, 0:400],
                                         start=False, stop=False,
                                         tile_position=tp)
                        nc.tensor.matmul(g_ps[r, 512:912], pa, ua[:, 512:912],
                                         start=False, stop=False,
                                         tile_position=tp)
                        nc.tensor.matmul(g_ps[r, 0:400], pb, ub[:, 0:400],
                                         start=False, stop=False,
                                         tile_position=tp)
                        nc.tensor.matmul(g_ps[r, 512:912], pb, ub[:, 512:912],
                                         start=False, stop=False,
                                         tile_position=tp)
                    # hrec_l[t-1] @ W_l
                    ra, rb = slots_a[l][sr[l]], slots_b[l][sr[l]]
                    nc.tensor.matmul(g_ps[r, 0:400], ra, w_a_s[l][:, 0:400],
                                     start=False, stop=False, tile_position=tp)
                    nc.tensor.matmul(g_ps[r, 512:912], ra, w_a_s[l][:, 512:912],
                                     start=False, stop=False, tile_position=tp)
                    nc.tensor.matmul(g_ps[r, 0:400], rb[0:KB, :],
                                     w_b_s[l][0:KB, 0:400],
                                     start=False, stop=True, tile_position=tp)
                    nc.tensor.matmul(g_ps[r, 512:912], rb[0:KB, :],
                                     w_b_s[l][0:KB, 512:912],
                                     start=False, stop=True, tile_position=tp)

                # merged gate math across all four layers ([0:104] rows)
                P = 32 * (L - 1) + BS  # 104
                sig_if = work.tile([P, 400], F32, name="sig_if")
                sig_o = work.tile([P, 200], F32, name="sig_o")
                tau_g = work.tile([P, 200], F32, name="tau_g")
                nc.scalar.activation(sig_if, g_ps[0:P, 0:400], AF.Sigmoid)
                nc.scalar.activation(tau_g, g_ps[0:P, 712:912], AF.Tanh)
                nc.scalar.activation(sig_o, g_ps[0:P, 512:712], AF.Sigmoid)
                m1 = work.tile([P, H], F32, name="m1")
                m2 = work.tile([P, H], F32, name="m2")
                nc.vector.tensor_mul(m1, sig_if[:, 200:400], c_s[0:P])
                nc.vector.tensor_mul(m2, sig_if[:, 0:200], tau_g)
                nc.vector.tensor_add(c_s[0:P], m1, m2)
                tau_c = work.tile([P, H], F32, name="tau_c")
                nc.scalar.activation(tau_c, c_s[0:P], AF.Tanh)
                h_s = work.tile([P, H], F32, name="h_s")
                nc.vector.tensor_mul(h_s, sig_o, tau_c)

                # transpose hrec into next slots (row-group concurrency)
                for l in range(L):
                    r = slice(32 * l, 32 * l + BS)
                    tpa = tps_pool.tile([KA, BS], F32, name="tpa")
                    tpb = tps_pool.tile([KB, BS], F32, name="tpb")
                    nc.tensor.transpose(tpa, h_s[r, 0:KA], i8_s[r],
                                        tile_position=(32 * l, 0))
                    nc.tensor.transpose(tpb, h_s[r, KA:H], i8_s[r],
                                        tile_position=(32 * l, 0))
                    if l % 2 == 0:
                        nc.vector.tensor_copy(slots_a[l][sw[l]], tpa)
                        nc.vector.tensor_copy(slots_b[l][sw[l]], tpb)
                    else:
                        nc.scalar.copy(slots_a[l][sw[l]], tpa)
                        nc.scalar.copy(slots_b[l][sw[l]], tpb)

                # logits for t = tau-3 (slot (u-3)%4 of every layer)
                ls = (u - 3) % 4
                lp = lps_pool.tile([BS, V], F32, name="l_ps")
                for l in range(L):
                    nc.tensor.matmul(lp, slots_a[l][ls], why_a_s[l],
                                     start=(l == 0), stop=False)
                    nc.tensor.matmul(lp, slots_b[l][ls], why_b_s[l][0:KB],
                                     start=False, stop=(l == L - 1))
                nc.scalar.copy(out_stage[u], lp)

            def flush_out(iv):
                for u in range(UNROLL):
                    nc.sync.dma_start(
                        out=lrec_dram[:].rearrange("t b v -> (t b) v")[
                            bass.ds((iv + u) * BS, BS)],
                        in_=out_stage[u])

            n_iter = (nT + UNROLL - 1) // UNROLL
            if static_scan:
                for it in range(n_iter):
                    iv = it * UNROLL
                    for u in range(UNROLL):
                        stage_pre(iv, u)
                    for u in range(UNROLL):
                        emit_tick(iv, u)
                    flush_out(iv)
            else:
                for u in range(UNROLL):
                    stage_pre(0, u)
                with tc.For_i(0, n_iter * UNROLL, UNROLL,
                              hint_engines=(mybir.EngineType.PE,
                                            mybir.EngineType.Activation,
                                            mybir.EngineType.DVE,
                                            mybir.EngineType.SP),
                              ) as iv:
                    for u in range(UNROLL):
                        emit_tick(iv, u)
                    # prefetch next iteration's PRE (same tiles, next rows)
                    for u in range(UNROLL):
                        stage_pre(iv + UNROLL, u)
                    flush_out(iv)

            # ================= P5: merge logits =================
            with tc.tile_pool(name="p5", bufs=3) as p5pool:
                lrec_flat = lrec_dram[:].rearrange("t b v -> (t b) v")
                lex_flat = lex_dram[:]
                for m in range(NCH):
                    a = p5pool.tile([128, V], F32, name="m_a")
                    bt = p5pool.tile([128, V], F32, name="m_b")
                    o = p5pool.tile([128, V], F32, name="m_o")
                    nc.sync.dma_start(
                        out=a, in_=lrec_flat[PADF * BS + m * 128:
                                             PADF * BS + (m + 1) * 128])
                    nc.sync.dma_start(out=bt, in_=lex_flat[m * 128:(m + 1) * 128])
                    nc.vector.tensor_add(o, a, bt)
                    nc.sync.dma_start(
                        out=logits_out[:].rearrange("t b v -> (t b) v")[
                            m * 128:(m + 1) * 128],
                        in_=o)

    split_excess_waits(nc)
    return nc


def host_prep(inputs):
    """Host-side weight/constant prep (weights-only compute + input encoding).
    Returns (shared input map, per-core input maps)."""
    inp = {k: np.asarray(v) for k, v in inputs.items()}
    xb = inp["xb"].astype(np.int64)

    Ws = [inp["W_in_w"]] + [inp["Wh_w"][n] for n in range(L - 1)]
    Us = [inp["Uh_w"][n] for n in range(L - 1)]
    Ubs = [inp["Uh_b"][n] for n in range(L - 1)]
    h0s = [inp["h0_in"]] + [inp["h0_h"][n] for n in range(L - 1)]
    c0s = [inp["c0_in"]] + [inp["c0_h"][n] for n in range(L - 1)]

    shared = {}
    shared["c_mat"] = inp["C"].astype(np.float32)
    # u0_aug: U_in | U_in_b | h0_in @ W_in   (permuted+padded)
    u0 = np.concatenate([inp["U_in_w"], inp["U_in_b"][None, :],
                         (h0s[0] @ Ws[0])[None, :]], axis=0)
    shared["u0_aug"] = _pad_gate_row(u0[:, PERM])
    shared["u_a"] = np.stack([_pad_gate_row(Us[n][:, PERM])[0:KA]
                              for n in range(3)])
    shared["u_b"] = np.stack([_pad_gate_row(Us[n][:, PERM])[KA:H]
                              for n in range(3)])
    wa, wb = [], []
    for l in range(L):
        wp = _pad_gate_row(Ws[l][:, PERM])
        bias = Ubs[l - 1] if l >= 1 else np.zeros(G, np.float32)
        aug = np.stack([_pad_gate_row(bias[PERM]),
                        _pad_gate_row((h0s[l] @ Ws[l])[PERM])])
        wa.append(wp[0:KA])
        wb.append(np.concatenate([wp[KA:H], aug], axis=0))
    shared["w_a"] = np.stack(wa)
    shared["w_b"] = np.stack(wb)
    shared["vh_w"] = inp["Vh_w"].astype(np.float32)
    shared["vh_b"] = inp["Vh_b"].astype(np.float32)
    shared["why_a"] = inp["Why"][:, 0:KA].astype(np.float32)
    wyb = np.zeros((L, KB + 1, V), np.float32)
    wyb[:, 0:KB] = inp["Why"][:, KA:H]
    wyb[1, KB] = inp["b_y"]
    shared["why_b"] = wyb
    c0m = np.zeros((128, H), np.float32)
    for l in range(L):
        c0m[32 * l : 32 * l + BS] = c0s[l][None, :]
    shared["c0m"] = c0m
    i8r = np.zeros((128, BS), np.float32)
    for l in range(L):
        i8r[32 * l : 32 * l + BS] = np.eye(BS, dtype=np.float32)
    shared["i8r"] = i8r
    pr = np.zeros(GP, np.float32)
    pr[0:200] = -40.0      # i
    pr[200:400] = 40.0     # f
    pr[512:712] = -40.0    # o
    shared["pad_blk"] = np.broadcast_to(
        pr, (2 * PADF + 2 * UNROLL, BS, GP)).copy()
    aug2 = np.zeros((2, TB), np.float32)
    aug2[0] = 1.0
    aug2[1, 0:BS] = 1.0
    shared["emb_aug"] = aug2
    exa2 = np.zeros((2, TB + BS), np.float32)
    exa2[0] = 1.0
    exa2[1, 0:BS] = 1.0
    shared["ex_aug"] = exa2

    per_core = []
    for c in range(NCORES):
        xs = xb[c * BS:(c + 1) * BS]             # [BS, T]
        oh = np.zeros((V, TB), np.float32)
        cols = np.arange(TB)
        oh[xs.T.reshape(-1), cols] = 1.0          # col j = t*BS+b
        m = dict(shared)
        m["oh_T"] = oh
        per_core.append(m)
    return per_core


_NC_CACHE = {}


def kernel(**inputs) -> np.ndarray:
    if "nc" not in _NC_CACHE:
        nT = int(os.environ.get("KDEBUG_NT", NT))
        static = bool(int(os.environ.get("KSTATIC", "0")))
        _NC_CACHE["nc"] = build_nc(nT=nT, static_scan=static)
    nc = _NC_CACHE["nc"]
    per_core = host_prep(inputs)
    res = run_bass_kernel_spmd(nc, per_core, core_ids=list(range(NCORES)))
    outs = []
    for c in range(NCORES):
        lg = res.results[c]["logits"]            # [T, BS, V]
        outs.append(np.transpose(lg, (1, 0, 2)))  # [BS, T, V]
    return np.concatenate(outs, axis=0).astype(np.float32)


if __name__ == "__main__":
    import reference as ref
    inputs = ref.setup_inputs()
    expected = np.asarray(ref.reference(**inputs))
    actual = kernel(**{k: np.asarray(v) for k, v in inputs.items()})
    err = np.abs(actual - expected).max()
    rel = err / np.abs(expected).max()
    print(f"absmax err {err:.4e}  rel {rel:.4e}")
